# revision 1
# baseline (speedup 1.0000x reference)
"""Trainium2 Bass kernel for the grouped contrastive loss.

Math: for anchors i and positives j restricted to the same
sensitive-attribute group g (size P),
    row(i,j) = S_ij - D * log E_ij
with S_ij = <p_i, p_j>/t and E_ij = sum_d exp(p_i[d] p_j[d] / t)
(the log-softmax max-shift cancels analytically), and
    loss = sum_i -1/(N P_i^2) * sum_{j in g(i)} row(i,j).

Strategy: sort points by group host-side so the same-group mask becomes
dense per-group blocks. Work = slots, each slot = (block of <=128 sorted
anchors, j-window of <=W columns of that block's group). Per slot, on
device (anchors on partitions as 32 packs of 4 anchors x 32 dims):
  - S via one fp32 matmul (lhsT = anchor points [32,128], rhs = window
    points [32,W]).
  - E via: DVE tensor_scalar broadcast-multiply (per-pack scalar column
    against 4x-replicated window points), ACT exp (batched 8 packs), and
    per-pack bf16 matmuls against shifted block-diagonal ones that
    accumulate the 32 exp rows of each anchor into its PSUM row.
  - Ln on ACT with accum_out gives sum_j log E per anchor for free.
Dummy rows/columns are weighted out host-side (w=0) or corrected by the
exact constant D*ln(D)*n_dummy per slot. The 8 cores run one SPMD
program over per-core input arrays; each returns a [128] partial that the
host sums.
"""

import math
import os
import sys

sys.path.insert(0, "/opt/trn_rl_repo")

import numpy as np
import ml_dtypes

import concourse.bacc as bacc
import concourse.bass as bass
import concourse.tile as tile
from concourse import mybir
from concourse.bass_utils import run_bass_kernel_spmd

N_CORES = 8
D = 32
PACKS = 32  # packs of 4 anchors per 128-anchor block

last_run_info = {}


def _install_drain_split_patch():
    # This walrus build rejects Drain instructions carrying more than one
    # semaphore wait ("Too many sync wait commands"). TileContext's exit
    # emits one kernel-tail Drain with a wait per outstanding logical
    # processor; split the extras across additional single-wait Drains on
    # the same engine (sequential waits are semantically identical).
    import concourse.tile as tile_mod

    if getattr(tile_mod.TileContext, "_drain_split_patched", False):
        return

    def _drain_and_barrier(self, tick_clock, wait_clock):
        nc = self.nc
        drain_inst = nc.sync.drain()
        wait_clock.add_sem_waits(
            drain_inst.ins,
            tile_mod.ScopedClock({None: tick_clock.global_clock}),
        )
        si = drain_inst.ins.sync_info
        if si is not None and si.on_wait is not None and len(si.on_wait) > 1:
            waits = list(si.on_wait)
            si.on_wait = [waits[0]]
            for w in waits[1:]:
                d2 = nc.sync.drain()
                si2 = d2.ins.sync_info
                if si2 is None:
                    d2.ins.sync_info = type(si)(on_wait=[w], on_update=[])
                else:
                    si2.on_wait = [w]

        nc.all_engine_barrier()
        assert self.sems is not None
        popped = nc._tile_sem_poison_stack.pop()
        assert popped is self._sem_poison
        nc.clear_and_free_semaphores(list(self.sems.allocated().values()))
        nc.all_engine_barrier()

    tile_mod.TileContext._drain_and_barrier = _drain_and_barrier
    tile_mod.TileContext._drain_split_patched = True


def _install_ntff_hook():
    # bass_utils' trace path under axon imports antenv.axon_hooks, which is
    # absent in this image; provide the ctypes-based hook it expects.
    import contextlib
    import ctypes
    import types

    if "antenv.axon_hooks" in sys.modules:
        return

    def _make_hook():
        try:
            lib = ctypes.CDLL("/opt/axon/libaxon_pjrt.so")
        except OSError:
            return None
        if not hasattr(lib, "axon_start_nrt_profile"):
            return None
        lib.axon_start_nrt_profile.argtypes = [
            ctypes.POINTER(ctypes.c_int64),
            ctypes.c_size_t,
        ]
        lib.axon_start_nrt_profile.restype = ctypes.c_int64
        lib.axon_stop_nrt_profile.argtypes = [ctypes.c_char_p]
        lib.axon_stop_nrt_profile.restype = ctypes.c_int64

        @contextlib.contextmanager
        def _hook_cm(output_dir, device_ids):
            import jax

            jax.devices()
            if device_ids:
                ids = (ctypes.c_int64 * len(device_ids))(*device_ids)
                rc = lib.axon_start_nrt_profile(ids, len(device_ids))
            else:
                rc = lib.axon_start_nrt_profile(None, 0)
            if rc != 0:
                raise RuntimeError(f"axon_start_nrt_profile rc={rc}")
            try:
                yield
            finally:
                n = lib.axon_stop_nrt_profile(str(output_dir).encode())
                if n < 0:
                    raise RuntimeError(f"axon_stop_nrt_profile rc={n}")

        return _hook_cm

    hook = _make_hook()
    mod = types.ModuleType("antenv.axon_hooks")
    mod.get_axon_ntff_profile_hook = lambda: hook
    mod.set_axon_ntff_profile_hook = lambda h: None
    sys.modules["antenv.axon_hooks"] = mod


def _plan(sa_sorted):
    """Slot plan from the sorted attribute vector.

    Returns (W, ntiles, slots_per_core) where each slot is
    (pos0, row_lo, row_hi, g0, g1, c0, L):
      pos0: first sorted-anchor position of the 128-anchor block
      [row_lo, row_hi): rows of the block belonging to group [g0, g1)
      [c0, c0+L): this slot's j-window (sorted positions) within the group
    or None for a dummy slot.
    """
    n = len(sa_sorted)
    assert n % 128 == 0
    n_blocks = n // 128
    bounds = [0]
    for i in range(1, n):
        if sa_sorted[i] != sa_sorted[i - 1]:
            bounds.append(i)
    bounds.append(n)

    jobs = []  # (pos0, row_lo, row_hi, g0, g1)
    for b in range(n_blocks):
        pos0 = b * 128
        for gi in range(len(bounds) - 1):
            g0, g1 = bounds[gi], bounds[gi + 1]
            lo = max(pos0, g0)
            hi = min(pos0 + 128, g1)
            if lo < hi:
                jobs.append((pos0, lo - pos0, hi - pos0, g0, g1))

    best = None
    for W in range(128, 513, 16):
        T = sum((g1 - g0 + W - 1) // W for (_, _, _, g0, g1) in jobs)
        ntiles = (T + N_CORES - 1) // N_CORES
        cost = ntiles * W
        if best is None or cost < best[0] or (cost == best[0] and W > best[1]):
            best = (cost, W, ntiles)
    _, W, ntiles = best

    slots = []
    for pos0, row_lo, row_hi, g0, g1 in jobs:
        for c0 in range(g0, g1, W):
            L = min(W, g1 - c0)
            slots.append((pos0, row_lo, row_hi, g0, g1, c0, L))

    per_core = [[] for _ in range(N_CORES)]
    for i, s in enumerate(slots):
        per_core[i % N_CORES].append(s)
    for c in range(N_CORES):
        while len(per_core[c]) < ntiles:
            per_core[c].append(None)
    return W, ntiles, per_core


def _build_program(W, ntiles):
    # Bacc (not raw Bass): its compile() runs generate_event_semaphores,
    # which splits multi-semaphore waits to satisfy the TRN2 one-wait-per-
    # instruction constraint this walrus build enforces.
    nc = bacc.Bacc(
        "TRN2", target_bir_lowering=False, debug=False, num_devices=N_CORES
    )
    f32 = mybir.dt.float32
    bf16 = mybir.dt.bfloat16

    rep4_d = nc.dram_tensor("rep4", [128, ntiles * W], f32, kind="ExternalInput").ap()
    rhsj_d = nc.dram_tensor("rhsj", [32, ntiles * W], f32, kind="ExternalInput").ap()
    lhsa_d = nc.dram_tensor("lhsa", [32, ntiles * 128], f32, kind="ExternalInput").ap()
    scal_d = nc.dram_tensor("scal", [128, ntiles * PACKS], f32, kind="ExternalInput").ap()
    wcol_d = nc.dram_tensor("wcol", [128, ntiles], f32, kind="ExternalInput").ap()
    kcol_d = nc.dram_tensor("kcol", [128, ntiles], f32, kind="ExternalInput").ap()
    ones_d = nc.dram_tensor("onesbd", [128, 8 * 32], bf16, kind="ExternalInput").ap()
    out_d = nc.dram_tensor("out", [128, 1], f32, kind="ExternalOutput").ap()

    Exp = mybir.ActivationFunctionType.Exp
    Ln = mybir.ActivationFunctionType.Ln

    with tile.TileContext(nc) as tc:
        with (
            tc.tile_pool(name="const", bufs=1) as cpool,
            tc.tile_pool(name="work", bufs=3) as wpool,
            tc.tile_pool(name="red", bufs=2) as rpool,
            tc.tile_pool(name="psE", bufs=2, space="PSUM") as psE,
            tc.tile_pool(name="psS", bufs=2, space="PSUM") as psS,
            tc.tile_pool(name="psL", bufs=1, space="PSUM") as psL,
        ):
            rep4 = cpool.tile([128, ntiles * W], f32, tag="rep4")
            nc.gpsimd.dma_start(rep4[:], rep4_d[:])
            rhsj = cpool.tile([32, ntiles * W], f32, tag="rhsj")
            nc.gpsimd.dma_start(rhsj[:], rhsj_d[:])
            lhsa = cpool.tile([32, ntiles * 128], f32, tag="lhsa")
            nc.gpsimd.dma_start(lhsa[:], lhsa_d[:])
            scal = cpool.tile([128, ntiles * PACKS], f32, tag="scal")
            nc.gpsimd.dma_start(scal[:], scal_d[:])
            wcol = cpool.tile([128, ntiles], f32, tag="wcol")
            nc.gpsimd.dma_start(wcol[:], wcol_d[:])
            kcol = cpool.tile([128, ntiles], f32, tag="kcol")
            nc.gpsimd.dma_start(kcol[:], kcol_d[:])
            onesbd = cpool.tile([128, 8 * 32], bf16, tag="onesbd")
            nc.gpsimd.dma_start(onesbd[:], ones_d[:])

            acc = cpool.tile([128, 1], f32, tag="acc")
            nc.vector.memset(acc[:], 0.0)

            for s in range(ntiles):
                S_ps = psS.tile([128, W], f32, tag="S")
                nc.tensor.matmul(
                    S_ps[:],
                    lhsT=lhsa[:, s * 128 : (s + 1) * 128],
                    rhs=rhsj[:, s * W : (s + 1) * W],
                    start=True,
                    stop=True,
                )
                # PSUM APs can only start at partition 0/32/64, so the 128
                # anchor rows of E live in two [64, W] tiles.
                E_lo = psE.tile([64, W], f32, tag="Elo")
                E_hi = psE.tile([64, W], f32, tag="Ehi")
                for h in range(4):
                    prod = wpool.tile([128, 8 * W], f32, tag="prod")
                    for i in range(8):
                        k = 8 * h + i
                        nc.vector.tensor_scalar_mul(
                            prod[:, i * W : (i + 1) * W],
                            rep4[:, s * W : (s + 1) * W],
                            scal[:, s * PACKS + k : s * PACKS + k + 1],
                        )
                    expt = wpool.tile([128, 8 * W], bf16, tag="expt")
                    nc.scalar.activation(expt[:], prod[:], Exp)
                    E_t = E_lo if h < 2 else E_hi
                    rb = 32 * (h % 2)
                    for i in range(8):
                        nc.tensor.matmul(
                            E_t[rb : rb + 32, :],
                            lhsT=onesbd[:, 32 * i : 32 * (i + 1)],
                            rhs=expt[:, i * W : (i + 1) * W],
                            start=(i == 0),
                            stop=(i == 7),
                        )
                logE = psL.tile([128, W], f32, tag="logE")
                sL = rpool.tile([128, 1], f32, tag="sL")
                nc.scalar.activation(logE[0:64, :], E_lo[:], Ln, accum_out=sL[0:64, :])
                nc.scalar.activation(logE[64:128, :], E_hi[:], Ln, accum_out=sL[64:128, :])
                sS = rpool.tile([128, 1], f32, tag="sS")
                nc.vector.tensor_reduce(
                    sS[:], S_ps[:], axis=mybir.AxisListType.X, op=mybir.AluOpType.add
                )
                v1 = rpool.tile([128, 1], f32, tag="v1")
                nc.vector.tensor_scalar(
                    v1[:],
                    sL[:],
                    -float(D),
                    kcol[:, s : s + 1],
                    op0=mybir.AluOpType.mult,
                    op1=mybir.AluOpType.add,
                )
                v2 = rpool.tile([128, 1], f32, tag="v2")
                nc.vector.tensor_add(v2[:], v1[:], sS[:])
                nc.vector.scalar_tensor_tensor(
                    acc[:],
                    v2[:],
                    wcol[:, s : s + 1],
                    acc[:],
                    op0=mybir.AluOpType.mult,
                    op1=mybir.AluOpType.add,
                )

            nc.gpsimd.dma_start(out_d[:], acc[:])

    nc.compile()
    return nc


def kernel(points, sensitive_attribute, t):
    _install_ntff_hook()

    points = np.asarray(points, dtype=np.float32)
    sa = np.asarray(sensitive_attribute).astype(np.int64)
    n, d = points.shape
    assert d == D

    scale = 1.0 / math.sqrt(float(np.asarray(t)))
    order = np.argsort(sa, kind="stable")
    sa_sorted = sa[order]
    ps = (points[order] * np.float32(scale)).astype(np.float32)  # [n, 32] sorted

    W, ntiles, per_core = _plan(sa_sorted)

    lnD = math.log(float(D))
    in_maps = []
    for c in range(N_CORES):
        rep4 = np.zeros((128, ntiles * W), np.float32)
        rhsj = np.zeros((32, ntiles * W), np.float32)
        lhsa = np.zeros((32, ntiles * 128), np.float32)
        scal = np.zeros((128, ntiles * PACKS), np.float32)
        wcol = np.zeros((128, ntiles), np.float32)
        kcol = np.zeros((128, ntiles), np.float32)
        for s, slot in enumerate(per_core[c]):
            if slot is None:
                # dummy slot: all-zero data; exp(0) rows sum to D, finite
                # log, zero weight. Correction value irrelevant (w=0).
                continue
            pos0, row_lo, row_hi, g0, g1, c0, L = slot
            P = g1 - g0
            win = ps[c0 : c0 + L].T  # [32, L]
            rhsj[:, s * W : s * W + L] = win
            rep4[:, s * W : s * W + L] = np.tile(win, (4, 1))
            ablk = np.zeros((32, 128), np.float32)
            ablk[:, row_lo:row_hi] = ps[pos0 + row_lo : pos0 + row_hi].T
            lhsa[:, s * 128 : (s + 1) * 128] = ablk
            # scal column k = anchors 4k..4k+3 flattened (a-major, d-minor)
            scal[:, s * PACKS : (s + 1) * PACKS] = (
                ablk.T.reshape(PACKS, 128).T
            )
            wcol[row_lo:row_hi, s] = -1.0 / (n * float(P) * float(P))
            kcol[:, s] = D * lnD * (W - L)

        onesbd = np.zeros((128, 8 * 32), ml_dtypes.bfloat16)
        for r in range(8):
            for a in range(4):
                onesbd[32 * a : 32 * (a + 1), 32 * r + 4 * r + a] = 1.0
        in_maps.append(
            {
                "rep4": rep4,
                "rhsj": rhsj,
                "lhsa": lhsa,
                "scal": scal,
                "wcol": wcol,
                "kcol": kcol,
                "onesbd": onesbd,
            }
        )

    nc = _build_program(W, ntiles)
    trace = bool(int(os.environ.get("KERNEL_TRACE", "0")))
    res = run_bass_kernel_spmd(nc, in_maps, list(range(N_CORES)), trace=trace)
    last_run_info["exec_time_ns"] = res.exec_time_ns
    last_run_info["mean_exec_time_ns"] = res.mean_exec_time_ns
    last_run_info["W"] = W
    last_run_info["ntiles"] = ntiles
    last_run_info["instructions"] = (
        res.instructions_and_trace[0] if res.instructions_and_trace else None
    )

    total = 0.0
    for c in range(N_CORES):
        total += float(res.results[c]["out"].astype(np.float64).sum())
    return np.float32(total)



# revision 2
# speedup vs baseline: 1.5746x; 1.5746x over previous
"""Trainium2 Bass kernel for the grouped contrastive loss.

Math: for anchors i and positives j restricted to the same
sensitive-attribute group g (size P),
    row(i,j) = S_ij - D * log E_ij
with S_ij = <p_i, p_j>/t and E_ij = sum_d exp(p_i[d] p_j[d] / t)
(the log-softmax max-shift cancels analytically), and
    loss = sum_i -1/(N P_i^2) * sum_{j in g(i)} row(i,j).

row(i,j) is symmetric, so the group's P x P matrix is covered by
chunking each group into <=128-column chunks: the diagonal chunk-square
is computed in full at weight 1 and cross chunk pairs only once (rows of
earlier chunks x cols of later chunk) at weight 2 -- B(B+1)/2 slots per
group instead of B^2. Slot = up to 128 anchor rows x one col chunk
(W=128), rows packed 128-at-a-time from all chunks <= the col chunk.

Per slot, on device (anchors on partitions as 32 packs of 4 anchors x
32 dims):
  - S via one bf16 matmul (lhsT = anchor points [32,128], rhs = window
    points [32,128]).
  - prod via 32 DVE tensor_scalar broadcast-multiplies (bf16 in/out,
    fp32 per-partition scalar: 4x DVE mode), one batched ACT Exp
    ([128, 4096] bf16), and per-pack bf16 matmuls against shifted
    block-diagonal ones accumulating each anchor's 32 exp rows into its
    PSUM row (4 chains x 8 packs into one [128,128] PSUM tile via
    explicit tile_position).
  - Ln on ACT, then DVE row-reductions of log E and S; weighted
    accumulate into a [128] per-core partial.
A manually pre-placed InstLoadActFuncSet of the combined exp+ln table
avoids the per-switch ACT table reloads. Dummy rows/cols are weighted
out (w=0) or corrected by the exact constant D*ln(D)*n_dummy per slot.
The 8 cores run one SPMD program; the host sums the [128] partials.
"""

import math
import os
import sys

sys.path.insert(0, "/opt/trn_rl_repo")

import numpy as np
import ml_dtypes

import concourse.bacc as bacc
import concourse.bass as bass
import concourse.tile as tile
from concourse import mybir
from concourse.bass_utils import run_bass_kernel_spmd

N_CORES = 8
D = 32
W = 128  # window (col chunk) width
PACKS = 32  # packs of 4 anchors per 128-anchor slot

last_run_info = {}


def _install_ntff_hook():
    # bass_utils' trace path under axon imports antenv.axon_hooks, which is
    # absent in this image; provide the ctypes-based hook it expects.
    import contextlib
    import ctypes
    import types

    if "antenv.axon_hooks" in sys.modules:
        return

    def _make_hook():
        try:
            lib = ctypes.CDLL("/opt/axon/libaxon_pjrt.so")
        except OSError:
            return None
        if not hasattr(lib, "axon_start_nrt_profile"):
            return None
        lib.axon_start_nrt_profile.argtypes = [
            ctypes.POINTER(ctypes.c_int64),
            ctypes.c_size_t,
        ]
        lib.axon_start_nrt_profile.restype = ctypes.c_int64
        lib.axon_stop_nrt_profile.argtypes = [ctypes.c_char_p]
        lib.axon_stop_nrt_profile.restype = ctypes.c_int64

        @contextlib.contextmanager
        def _hook_cm(output_dir, device_ids):
            import jax

            jax.devices()
            if device_ids:
                ids = (ctypes.c_int64 * len(device_ids))(*device_ids)
                rc = lib.axon_start_nrt_profile(ids, len(device_ids))
            else:
                rc = lib.axon_start_nrt_profile(None, 0)
            if rc != 0:
                raise RuntimeError(f"axon_start_nrt_profile rc={rc}")
            try:
                yield
            finally:
                n = lib.axon_stop_nrt_profile(str(output_dir).encode())
                if n < 0:
                    raise RuntimeError(f"axon_stop_nrt_profile rc={n}")

        return _hook_cm

    hook = _make_hook()
    mod = types.ModuleType("antenv.axon_hooks")
    mod.get_axon_ntff_profile_hook = lambda: hook
    mod.set_axon_ntff_profile_hook = lambda h: None
    sys.modules["antenv.axon_hooks"] = mod


def _plan(sa_sorted):
    """Slot plan from the sorted attribute vector.

    Each slot is (rows, weights, c0, L):
      rows: array of <=128 sorted-anchor positions (the slot's anchors)
      weights: per-row pair multiplicity (1 diag chunk, 2 earlier chunk)
      [c0, c0+L): the slot's col window (sorted positions, one chunk)
    or None for a dummy slot. Returns (ntiles, per_core).
    """
    n = len(sa_sorted)
    bounds = [0]
    for i in range(1, n):
        if sa_sorted[i] != sa_sorted[i - 1]:
            bounds.append(i)
    bounds.append(n)

    slots = []
    for gi in range(len(bounds) - 1):
        g0, g1 = bounds[gi], bounds[gi + 1]
        P = g1 - g0
        B = (P + W - 1) // W
        for w in range(B):
            c0 = g0 + W * w
            L = min(W, g1 - c0)
            r_hi = min(g0 + W * (w + 1), g1)  # rows of chunks 0..w
            rows_all = np.arange(g0, r_hi)
            wts_all = np.where(rows_all < c0, 2.0, 1.0)
            for r0 in range(0, len(rows_all), 128):
                slots.append(
                    (rows_all[r0 : r0 + 128], wts_all[r0 : r0 + 128], c0, L)
                )

    ntiles = (len(slots) + N_CORES - 1) // N_CORES
    per_core = [[] for _ in range(N_CORES)]
    for i, s in enumerate(slots):
        per_core[i % N_CORES].append(s)
    for c in range(N_CORES):
        while len(per_core[c]) < ntiles:
            per_core[c].append(None)
    return ntiles, per_core


def _exp_ln_table_id(nc):
    try:
        from concourse.hw_specs import get_activation_tables

        tabs = get_activation_tables(nc.m.arch)
        Exp = mybir.ActivationFunctionType.Exp
        Ln = mybir.ActivationFunctionType.Ln
        for idx, funcs in enumerate(tabs.values()):
            if Exp in funcs and Ln in funcs:
                return idx
    except Exception:
        pass
    return 6  # natural_log_exp_and_others in this neuronxcc's act_info.json


def _build_program(ntiles):
    # Bacc (not raw Bass): its compile() runs generate_event_semaphores,
    # which splits multi-semaphore waits to satisfy the TRN2 one-wait-per-
    # instruction constraint this walrus build enforces.
    nc = bacc.Bacc(
        "TRN2", target_bir_lowering=False, debug=False, num_devices=N_CORES
    )
    f32 = mybir.dt.float32
    bf16 = mybir.dt.bfloat16

    rep4_d = nc.dram_tensor("rep4", [128, ntiles * W], bf16, kind="ExternalInput").ap()
    winj_d = nc.dram_tensor("winj", [32, ntiles * W], bf16, kind="ExternalInput").ap()
    lhsa_d = nc.dram_tensor("lhsa", [32, ntiles * 128], bf16, kind="ExternalInput").ap()
    scal_d = nc.dram_tensor("scal", [128, ntiles * PACKS], f32, kind="ExternalInput").ap()
    wcol_d = nc.dram_tensor("wcol", [128, ntiles], f32, kind="ExternalInput").ap()
    kcol_d = nc.dram_tensor("kcol", [128, ntiles], f32, kind="ExternalInput").ap()
    ones_d = nc.dram_tensor("onesbd", [128, 8 * 32], bf16, kind="ExternalInput").ap()
    out_d = nc.dram_tensor("out", [128, 1], f32, kind="ExternalOutput").ap()

    Exp = mybir.ActivationFunctionType.Exp
    Ln = mybir.ActivationFunctionType.Ln

    with tile.TileContext(nc) as tc:
        with (
            tc.tile_pool(name="const", bufs=1) as cpool,
            tc.tile_pool(name="work", bufs=2) as wpool,
            tc.tile_pool(name="red", bufs=2) as rpool,
            tc.tile_pool(name="psE", bufs=2, space="PSUM") as psE,
            tc.tile_pool(name="psS", bufs=2, space="PSUM") as psS,
            tc.tile_pool(name="psL", bufs=2, space="PSUM") as psL,
        ):
            # preload the combined exp+ln table so Exp/Ln interleaving
            # never reloads activation tables (saves ~1.3us per switch)
            nc.scalar.add_instruction(
                mybir.InstLoadActFuncSet(
                    name=nc.get_next_instruction_name(),
                    ins=[],
                    outs=[],
                    act_func_set_id=_exp_ln_table_id(nc),
                )
            )

            rep4 = cpool.tile([128, ntiles * W], bf16, tag="rep4")
            nc.gpsimd.dma_start(rep4[:], rep4_d[:])
            scal = cpool.tile([128, ntiles * PACKS], f32, tag="scal")
            nc.gpsimd.dma_start(scal[:], scal_d[:])
            winj = cpool.tile([32, ntiles * W], bf16, tag="winj")
            nc.sync.dma_start(winj[:], winj_d[:])
            lhsa = cpool.tile([32, ntiles * 128], bf16, tag="lhsa")
            nc.sync.dma_start(lhsa[:], lhsa_d[:])
            onesbd = cpool.tile([128, 8 * 32], bf16, tag="onesbd")
            nc.sync.dma_start(onesbd[:], ones_d[:])
            wcol = cpool.tile([128, ntiles], f32, tag="wcol")
            nc.gpsimd.dma_start(wcol[:], wcol_d[:])
            kcol = cpool.tile([128, ntiles], f32, tag="kcol")
            nc.gpsimd.dma_start(kcol[:], kcol_d[:])

            acc = cpool.tile([128, 1], f32, tag="acc")
            nc.vector.memset(acc[:], 0.0)

            for s in range(ntiles):
                S_ps = psS.tile([128, W], f32, tag="S")
                nc.tensor.matmul(
                    S_ps[:],
                    lhsT=lhsa[:, s * 128 : (s + 1) * 128],
                    rhs=winj[:, s * W : (s + 1) * W],
                    start=True,
                    stop=True,
                )
                prod = wpool.tile([128, PACKS * W], bf16, tag="prod")
                for k in range(PACKS):
                    nc.vector.tensor_scalar_mul(
                        prod[:, k * W : (k + 1) * W],
                        rep4[:, s * W : (s + 1) * W],
                        scal[:, s * PACKS + k : s * PACKS + k + 1],
                    )
                expt = wpool.tile([128, PACKS * W], bf16, tag="expt")
                nc.scalar.activation(expt[:], prod[:], Exp)
                E_ps = psE.tile([128, W], f32, tag="E")
                for h in range(4):
                    for i in range(8):
                        k = 8 * h + i
                        nc.tensor.matmul(
                            E_ps[32 * h : 32 * h + 32, :],
                            lhsT=onesbd[:, 32 * i : 32 * (i + 1)],
                            rhs=expt[:, k * W : (k + 1) * W],
                            start=(i == 0),
                            stop=(i == 7),
                            tile_position=(0, 32 * h),
                        )
                logE = psL.tile([128, W], f32, tag="logE")
                nc.scalar.activation(logE[:], E_ps[:], Ln)
                sL = rpool.tile([128, 1], f32, tag="sL")
                nc.vector.tensor_reduce(
                    sL[:], logE[:], axis=mybir.AxisListType.X, op=mybir.AluOpType.add
                )
                sS = rpool.tile([128, 1], f32, tag="sS")
                nc.vector.tensor_reduce(
                    sS[:], S_ps[:], axis=mybir.AxisListType.X, op=mybir.AluOpType.add
                )
                v1 = rpool.tile([128, 1], f32, tag="v1")
                nc.vector.tensor_scalar(
                    v1[:],
                    sL[:],
                    -float(D),
                    kcol[:, s : s + 1],
                    op0=mybir.AluOpType.mult,
                    op1=mybir.AluOpType.add,
                )
                v2 = rpool.tile([128, 1], f32, tag="v2")
                nc.vector.tensor_add(v2[:], v1[:], sS[:])
                nc.vector.scalar_tensor_tensor(
                    acc[:],
                    v2[:],
                    wcol[:, s : s + 1],
                    acc[:],
                    op0=mybir.AluOpType.mult,
                    op1=mybir.AluOpType.add,
                )

            nc.gpsimd.dma_start(out_d[:], acc[:])

    nc.compile()
    return nc


def kernel(points, sensitive_attribute, t):
    _install_ntff_hook()

    points = np.asarray(points, dtype=np.float32)
    sa = np.asarray(sensitive_attribute).astype(np.int64)
    n, d = points.shape
    assert d == D

    scale = 1.0 / math.sqrt(float(np.asarray(t)))
    order = np.argsort(sa, kind="stable")
    sa_sorted = sa[order]
    ps = (points[order] * np.float32(scale)).astype(np.float32)  # [n, 32] sorted
    ps_bf = ps.astype(ml_dtypes.bfloat16)

    # group size per sorted position (for the 1/P^2 weights)
    _, counts = np.unique(sa_sorted, return_counts=True)
    gsize = np.repeat(counts, counts).astype(np.float64)

    ntiles, per_core = _plan(sa_sorted)

    lnD = math.log(float(D))
    onesbd = np.zeros((128, 8 * 32), ml_dtypes.bfloat16)
    for i in range(8):
        for a in range(4):
            onesbd[32 * a : 32 * (a + 1), 32 * i + 4 * i + a] = 1.0

    in_maps = []
    for c in range(N_CORES):
        rep4 = np.zeros((128, ntiles * W), ml_dtypes.bfloat16)
        winj = np.zeros((32, ntiles * W), ml_dtypes.bfloat16)
        lhsa = np.zeros((32, ntiles * 128), ml_dtypes.bfloat16)
        scal = np.zeros((128, ntiles * PACKS), np.float32)
        wcol = np.zeros((128, ntiles), np.float32)
        kcol = np.zeros((128, ntiles), np.float32)
        for s, slot in enumerate(per_core[c]):
            if slot is None:
                # dummy slot: all-zero data; exp(0) rows sum to D, finite
                # log, zero weight.
                continue
            rows, wts, c0, L = slot
            R = len(rows)
            win = ps_bf[c0 : c0 + L].T  # [32, L]
            winj[:, s * W : s * W + L] = win
            rep4[:, s * W : s * W + L] = np.tile(win, (4, 1))
            ablk = np.zeros((32, 128), np.float32)
            ablk[:, :R] = ps[rows].T
            lhsa[:, s * 128 : (s + 1) * 128] = ablk.astype(ml_dtypes.bfloat16)
            # scal column k = slot rows 4k..4k+3 flattened (a-major, d-minor)
            scal[:, s * PACKS : (s + 1) * PACKS] = ablk.T.reshape(PACKS, 128).T
            P = gsize[rows]
            wcol[:R, s] = -wts / (n * P * P)
            kcol[:, s] = D * lnD * (W - L)

        in_maps.append(
            {
                "rep4": rep4,
                "winj": winj,
                "lhsa": lhsa,
                "scal": scal,
                "wcol": wcol,
                "kcol": kcol,
                "onesbd": onesbd,
            }
        )

    nc = _build_program(ntiles)
    trace = bool(int(os.environ.get("KERNEL_TRACE", "0")))
    res = run_bass_kernel_spmd(nc, in_maps, list(range(N_CORES)), trace=trace)
    last_run_info["exec_time_ns"] = res.exec_time_ns
    last_run_info["mean_exec_time_ns"] = res.mean_exec_time_ns
    last_run_info["W"] = W
    last_run_info["ntiles"] = ntiles
    last_run_info["instructions"] = (
        res.instructions_and_trace[0] if res.instructions_and_trace else None
    )

    total = 0.0
    for c in range(N_CORES):
        total += float(res.results[c]["out"].astype(np.float64).sum())
    return np.float32(total)


# revision 9
# speedup vs baseline: 2.0413x; 1.2964x over previous
"""Trainium2 Bass kernel for the grouped contrastive loss.

Math: for anchors i and positives j restricted to the same
sensitive-attribute group g (size P),
    row(i,j) = S_ij - D * log E_ij
with S_ij = <p_i, p_j>/t and E_ij = sum_d exp(p_i[d] p_j[d] / t)
(the log-softmax max-shift cancels analytically), and
    loss = sum_i -1/(N P_i^2) * sum_{j in g(i)} row(i,j).

row(i,j) is symmetric, so the group's P x P matrix is covered by
chunking each group into <=128-column chunks: the diagonal chunk-square
is computed in full at weight 1 and cross chunk pairs only once (rows of
earlier chunks x cols of later chunk) at weight 2 -- B(B+1)/2 slots per
group instead of B^2. Slot = up to 128 anchor rows x one col chunk
(W=128), rows packed 128-at-a-time from all chunks <= the col chunk.

Per slot, on device (anchors on partitions as 32 packs of 4 anchors x
32 dims):
  - S via one bf16 matmul (lhsT = anchor points [32,128], rhs = window
    points [32,128]).
  - prod via ONE DVE tensor_tensor per slot: scalars stored duplicated
    in pairs (scal2[p,2k]=scal2[p,2k+1]) so all three operands' APs end
    in a packed [1,2] bf16 dim -> DVE 2x mode; stride-0 outer dims do
    the pack/window broadcast. Then one batched ACT Exp ([128, 4096]
    bf16), and per-pack bf16 matmuls against shifted block-diagonal
    ones accumulating each anchor's 32 exp rows into its PSUM row
    (4 chains x 8 packs into one [128,128] PSUM tile via explicit
    tile_position).
  - Ln on ACT, then DVE row-reductions of log E and S; weighted
    accumulate into a [128] per-core partial.
A manually pre-placed InstLoadActFuncSet of the combined exp+ln table
avoids the per-switch ACT table reloads. Dummy rows/cols are weighted
out (w=0) or corrected by the exact constant D*ln(D)*n_dummy per slot.
The 8 cores run one SPMD program; the host sums the [128] partials.
"""

import math
import os
import sys

sys.path.insert(0, "/opt/trn_rl_repo")

import numpy as np
import ml_dtypes

import concourse.bacc as bacc
import concourse.bass as bass
import concourse.tile as tile
from concourse import mybir
from concourse.bass_utils import run_bass_kernel_spmd

N_CORES = 8
D = 32
W = 128  # window (col chunk) width
PACKS = 32  # packs of 4 anchors per 128-anchor slot

last_run_info = {}


def _install_ntff_hook():
    # bass_utils' trace path under axon imports antenv.axon_hooks, which is
    # absent in this image; provide the ctypes-based hook it expects.
    import contextlib
    import ctypes
    import types

    if "antenv.axon_hooks" in sys.modules:
        return

    def _make_hook():
        try:
            lib = ctypes.CDLL("/opt/axon/libaxon_pjrt.so")
        except OSError:
            return None
        if not hasattr(lib, "axon_start_nrt_profile"):
            return None
        lib.axon_start_nrt_profile.argtypes = [
            ctypes.POINTER(ctypes.c_int64),
            ctypes.c_size_t,
        ]
        lib.axon_start_nrt_profile.restype = ctypes.c_int64
        lib.axon_stop_nrt_profile.argtypes = [ctypes.c_char_p]
        lib.axon_stop_nrt_profile.restype = ctypes.c_int64

        @contextlib.contextmanager
        def _hook_cm(output_dir, device_ids):
            import jax

            jax.devices()
            if device_ids:
                ids = (ctypes.c_int64 * len(device_ids))(*device_ids)
                rc = lib.axon_start_nrt_profile(ids, len(device_ids))
            else:
                rc = lib.axon_start_nrt_profile(None, 0)
            if rc != 0:
                raise RuntimeError(f"axon_start_nrt_profile rc={rc}")
            try:
                yield
            finally:
                n = lib.axon_stop_nrt_profile(str(output_dir).encode())
                if n < 0:
                    raise RuntimeError(f"axon_stop_nrt_profile rc={n}")

        return _hook_cm

    hook = _make_hook()
    mod = types.ModuleType("antenv.axon_hooks")
    mod.get_axon_ntff_profile_hook = lambda: hook
    mod.set_axon_ntff_profile_hook = lambda h: None
    sys.modules["antenv.axon_hooks"] = mod


def _plan(sa_sorted):
    """Slot plan from the sorted attribute vector.

    Each slot is (rows, weights, c0, L):
      rows: array of <=128 sorted-anchor positions (the slot's anchors)
      weights: per-row pair multiplicity (1 diag chunk, 2 earlier chunk)
      [c0, c0+L): the slot's col window (sorted positions, one chunk)
    or None for a dummy slot. Returns (ntiles, per_core).
    """
    n = len(sa_sorted)
    bounds = [0]
    for i in range(1, n):
        if sa_sorted[i] != sa_sorted[i - 1]:
            bounds.append(i)
    bounds.append(n)

    slots = []
    for gi in range(len(bounds) - 1):
        g0, g1 = bounds[gi], bounds[gi + 1]
        P = g1 - g0
        B = (P + W - 1) // W
        for w in range(B):
            c0 = g0 + W * w
            L = min(W, g1 - c0)
            r_hi = min(g0 + W * (w + 1), g1)  # rows of chunks 0..w
            rows_all = np.arange(g0, r_hi)
            wts_all = np.where(rows_all < c0, 2.0, 1.0)
            for r0 in range(0, len(rows_all), 128):
                slots.append(
                    (rows_all[r0 : r0 + 128], wts_all[r0 : r0 + 128], c0, L)
                )

    ntiles = (len(slots) + N_CORES - 1) // N_CORES
    per_core = [[] for _ in range(N_CORES)]
    for i, s in enumerate(slots):
        per_core[i % N_CORES].append(s)
    for c in range(N_CORES):
        while len(per_core[c]) < ntiles:
            per_core[c].append(None)
    return ntiles, per_core


def _exp_ln_table_id(nc):
    try:
        from concourse.hw_specs import get_activation_tables

        tabs = get_activation_tables(nc.m.arch)
        Exp = mybir.ActivationFunctionType.Exp
        Ln = mybir.ActivationFunctionType.Ln
        for idx, funcs in enumerate(tabs.values()):
            if Exp in funcs and Ln in funcs:
                return idx
    except Exception:
        pass
    return 6  # natural_log_exp_and_others in this neuronxcc's act_info.json


def _build_program(ntiles):
    # Bacc (not raw Bass): its compile() runs generate_event_semaphores,
    # which splits multi-semaphore waits to satisfy the TRN2 one-wait-per-
    # instruction constraint this walrus build enforces.
    nc = bacc.Bacc(
        "TRN2", target_bir_lowering=False, debug=False, num_devices=N_CORES
    )
    f32 = mybir.dt.float32
    bf16 = mybir.dt.bfloat16

    rep4_d = nc.dram_tensor("rep4", [128, ntiles * W], bf16, kind="ExternalInput").ap()
    winj_d = nc.dram_tensor("winj", [32, ntiles * W], bf16, kind="ExternalInput").ap()
    lhsa_d = nc.dram_tensor("lhsa", [32, ntiles * 128], bf16, kind="ExternalInput").ap()
    scal_d = nc.dram_tensor(
        "scal2", [128, ntiles * 2 * PACKS], bf16, kind="ExternalInput"
    ).ap()
    wcol_d = nc.dram_tensor("wcol", [128, ntiles], f32, kind="ExternalInput").ap()
    kcol_d = nc.dram_tensor("kcol", [128, ntiles], f32, kind="ExternalInput").ap()
    ones_d = nc.dram_tensor("onesbd", [128, 8 * 32], bf16, kind="ExternalInput").ap()
    out_d = nc.dram_tensor("out", [128, 1], f32, kind="ExternalOutput").ap()

    Exp = mybir.ActivationFunctionType.Exp
    Ln = mybir.ActivationFunctionType.Ln

    with tile.TileContext(nc) as tc:
        with (
            tc.tile_pool(name="const", bufs=1) as cpool,
            tc.tile_pool(name="work", bufs=2) as wpool,
            tc.tile_pool(name="red", bufs=2) as rpool,
            tc.tile_pool(name="psE", bufs=2, space="PSUM") as psE,
            tc.tile_pool(name="psS", bufs=2, space="PSUM") as psS,
            tc.tile_pool(name="psL", bufs=2, space="PSUM") as psL,
        ):
            # preload the combined exp+ln table so Exp/Ln interleaving
            # never reloads activation tables (saves ~1.3us per switch)
            nc.scalar.add_instruction(
                mybir.InstLoadActFuncSet(
                    name=nc.get_next_instruction_name(),
                    ins=[],
                    outs=[],
                    act_func_set_id=_exp_ln_table_id(nc),
                )
            )

            # slot-0 slices land first so compute starts before the bulk
            rep4 = cpool.tile([128, ntiles * W], bf16, tag="rep4")
            nc.gpsimd.dma_start(rep4[:, 0:W], rep4_d[:, 0:W])
            scal = cpool.tile([128, ntiles * 2 * PACKS], bf16, tag="scal2")
            nc.gpsimd.dma_start(scal[:, 0 : 2 * PACKS], scal_d[:, 0 : 2 * PACKS])
            nc.gpsimd.dma_start(rep4[:, W:], rep4_d[:, W:])
            nc.gpsimd.dma_start(scal[:, 2 * PACKS :], scal_d[:, 2 * PACKS :])
            winj = cpool.tile([32, ntiles * W], bf16, tag="winj")
            nc.sync.dma_start(winj[:], winj_d[:])
            lhsa = cpool.tile([32, ntiles * 128], bf16, tag="lhsa")
            nc.sync.dma_start(lhsa[:], lhsa_d[:])
            onesbd = cpool.tile([128, 8 * 32], bf16, tag="onesbd")
            nc.sync.dma_start(onesbd[:], ones_d[:])
            wcol = cpool.tile([128, ntiles], f32, tag="wcol")
            nc.gpsimd.dma_start(wcol[:], wcol_d[:])
            kcol = cpool.tile([128, ntiles], f32, tag="kcol")
            nc.gpsimd.dma_start(kcol[:], kcol_d[:])

            acc = cpool.tile([128, 1], f32, tag="acc")
            nc.vector.memset(acc[:], 0.0)

            for s in range(ntiles):
                S_ps = psS.tile([128, W], f32, tag="S")
                nc.tensor.matmul(
                    S_ps[:],
                    lhsT=lhsa[:, s * 128 : (s + 1) * 128],
                    rhs=winj[:, s * W : (s + 1) * W],
                    start=True,
                    stop=True,
                )
                prod = wpool.tile([128, PACKS * W], bf16, tag="prod")
                in0 = (
                    rep4[:, s * W : (s + 1) * W]
                    .rearrange("p (j2 two) -> p j2 two", two=2)
                    .unsqueeze(1)
                    .broadcast_to([128, PACKS, W // 2, 2])
                )
                in1 = (
                    scal[:, s * 2 * PACKS : (s + 1) * 2 * PACKS]
                    .rearrange("p (k two) -> p k two", two=2)
                    .unsqueeze(2)
                    .broadcast_to([128, PACKS, W // 2, 2])
                )
                outp = prod[:].rearrange(
                    "p (k j2 two) -> p k j2 two", k=PACKS, two=2
                )
                nc.vector.tensor_tensor(outp, in0, in1, op=mybir.AluOpType.mult)
                expt = wpool.tile([128, PACKS * W], bf16, tag="expt")
                nc.scalar.activation(expt[:], prod[:], Exp)
                E_ps = psE.tile([128, W], f32, tag="E")
                for h in range(4):
                    for i in range(8):
                        k = 8 * h + i
                        nc.tensor.matmul(
                            E_ps[32 * h : 32 * h + 32, :],
                            lhsT=onesbd[:, 32 * i : 32 * (i + 1)],
                            rhs=expt[:, k * W : (k + 1) * W],
                            start=(i == 0),
                            stop=(i == 7),
                            tile_position=(0, 32 * h),
                        )
                logE = psL.tile([128, W], f32, tag="logE")
                nc.scalar.activation(logE[:], E_ps[:], Ln)
                sL = rpool.tile([128, 1], f32, tag="sL")
                nc.vector.tensor_reduce(
                    sL[:], logE[:], axis=mybir.AxisListType.X, op=mybir.AluOpType.add
                )
                sS = rpool.tile([128, 1], f32, tag="sS")
                nc.vector.tensor_reduce(
                    sS[:], S_ps[:], axis=mybir.AxisListType.X, op=mybir.AluOpType.add
                )
                v1 = rpool.tile([128, 1], f32, tag="v1")
                nc.vector.tensor_scalar(
                    v1[:],
                    sL[:],
                    -float(D),
                    kcol[:, s : s + 1],
                    op0=mybir.AluOpType.mult,
                    op1=mybir.AluOpType.add,
                )
                v2 = rpool.tile([128, 1], f32, tag="v2")
                nc.vector.tensor_add(v2[:], v1[:], sS[:])
                nc.vector.scalar_tensor_tensor(
                    acc[:],
                    v2[:],
                    wcol[:, s : s + 1],
                    acc[:],
                    op0=mybir.AluOpType.mult,
                    op1=mybir.AluOpType.add,
                )

            nc.gpsimd.dma_start(out_d[:], acc[:])

    nc.compile()
    return nc


def kernel(points, sensitive_attribute, t):
    _install_ntff_hook()

    points = np.asarray(points, dtype=np.float32)
    sa = np.asarray(sensitive_attribute).astype(np.int64)
    n, d = points.shape
    assert d == D

    scale = 1.0 / math.sqrt(float(np.asarray(t)))
    order = np.argsort(sa, kind="stable")
    sa_sorted = sa[order]
    ps = (points[order] * np.float32(scale)).astype(np.float32)  # [n, 32] sorted
    ps_bf = ps.astype(ml_dtypes.bfloat16)

    # group size per sorted position (for the 1/P^2 weights)
    _, counts = np.unique(sa_sorted, return_counts=True)
    gsize = np.repeat(counts, counts).astype(np.float64)

    ntiles, per_core = _plan(sa_sorted)

    lnD = math.log(float(D))
    onesbd = np.zeros((128, 8 * 32), ml_dtypes.bfloat16)
    for i in range(8):
        for a in range(4):
            onesbd[32 * a : 32 * (a + 1), 32 * i + 4 * i + a] = 1.0

    in_maps = []
    for c in range(N_CORES):
        rep4 = np.zeros((128, ntiles * W), ml_dtypes.bfloat16)
        winj = np.zeros((32, ntiles * W), ml_dtypes.bfloat16)
        lhsa = np.zeros((32, ntiles * 128), ml_dtypes.bfloat16)
        scal2 = np.zeros((128, ntiles * 2 * PACKS), ml_dtypes.bfloat16)
        wcol = np.zeros((128, ntiles), np.float32)
        kcol = np.zeros((128, ntiles), np.float32)
        for s, slot in enumerate(per_core[c]):
            if slot is None:
                # dummy slot: all-zero data; exp(0) rows sum to D, finite
                # log, zero weight.
                continue
            rows, wts, c0, L = slot
            R = len(rows)
            win = ps_bf[c0 : c0 + L].T  # [32, L]
            winj[:, s * W : s * W + L] = win
            rep4[:, s * W : s * W + L] = np.tile(win, (4, 1))
            ablk = np.zeros((32, 128), np.float32)
            ablk[:, :R] = ps[rows].T
            lhsa[:, s * 128 : (s + 1) * 128] = ablk.astype(ml_dtypes.bfloat16)
            # scal column k = slot rows 4k..4k+3 flattened (a-major,
            # d-minor), stored twice (pair duplication for DVE 2x)
            sc = ablk.T.reshape(PACKS, 128).T.astype(ml_dtypes.bfloat16)
            scal2[:, s * 2 * PACKS : (s + 1) * 2 * PACKS] = np.repeat(sc, 2, axis=1)
            P = gsize[rows]
            wcol[:R, s] = -wts / (n * P * P)
            kcol[:, s] = D * lnD * (W - L)

        in_maps.append(
            {
                "rep4": rep4,
                "winj": winj,
                "lhsa": lhsa,
                "scal2": scal2,
                "wcol": wcol,
                "kcol": kcol,
                "onesbd": onesbd,
            }
        )

    nc = _build_program(ntiles)
    trace = bool(int(os.environ.get("KERNEL_TRACE", "0")))
    res = run_bass_kernel_spmd(nc, in_maps, list(range(N_CORES)), trace=trace)
    last_run_info["exec_time_ns"] = res.exec_time_ns
    last_run_info["mean_exec_time_ns"] = res.mean_exec_time_ns
    last_run_info["W"] = W
    last_run_info["ntiles"] = ntiles
    last_run_info["instructions"] = (
        res.instructions_and_trace[0] if res.instructions_and_trace else None
    )

    total = 0.0
    for c in range(N_CORES):
        total += float(res.results[c]["out"].astype(np.float64).sum())
    return np.float32(total)
